# revision 2
# baseline (speedup 1.0000x reference)
"""GATv2 2-layer GNN on 8 Trainium2 NeuronCores - instruction-minimal design.

On this stack each instruction costs ~30-50us regardless of operand size and
dma_gather costs ~0.25us/index, so the kernel minimizes instruction count and
gather-index count:
  - edges processed in 7 chunks (7 dst-windows of 128 dsts each); every
    elementwise step is ONE batched op per chunk
  - per (dst, table-half) edge groups padded to x4 ("quads"); a 2-level DVE
    pair-reduction collapses 4 edge rows -> 1 before the PE segment matmuls
    (4x fewer matmuls). Quad members sit in the same partition across 4
    consecutive gather tiles.
  - xr rows gathered once per quad (not per edge)
  - pad edges gather a special table row holding -1e4*sign(att), which drives
    their logit e to -huge so exp(e) is exactly 0: no masking needed
  - segment softmax skips max-subtraction (e stays within f16-exp range here)
  - aggregation via indicator matmuls into one [128,7,512] PSUM tile per
    chunk, finalized with a handful of batched ops
  - appended self-loops bypass the edge pipeline entirely (their exact
    per-dst contribution is added during finalize)
"""
import sys

sys.path.insert(0, "/opt/trn_rl_repo")

import numpy as np

N = 50000
IN, HID, H, OUT = 128, 64, 4, 64
SLOPE = 0.2
NC = 8
NLOC = N // NC            # 6250
NPAD = 6272               # 49*128
NWIN = NPAD // 128        # 49
NROWS = NPAD * NC         # 50176
HALF = NROWS // 2         # 25088
WPC = 7                   # windows per chunk
NCHUNK = NWIN // WPC      # 7
CAP = 12                  # tiles per dma_gather call
SENT = 2000               # seg sentinel (matches no indicator column)

_cache = {}


def _wrap16(stream):
    n = len(stream)
    a = np.zeros((16, n // 16), np.int16)
    a[np.arange(n) % 16, np.arange(n) // 16] = stream
    return np.tile(a, (8, 1))


def _host_metadata(edge_index):
    """Chunk metadata. Table layout per layer (NROWS+2 rows):
    row 0 = padA, rows 1..HALF = nodes 0..HALF-1 (A idx = srow+1),
    rows HALF+1..NROWS = nodes HALF.. (B idx = srow-HALF), row NROWS+1 =
    padB (B idx = HALF)."""
    src = np.asarray(edge_index[0], np.int64)
    dst = np.asarray(edge_index[1], np.int64)
    srow = (src // NLOC) * NPAD + (src % NLOC)
    dcore = dst // NLOC
    dloc = dst % NLOC

    per_core = []
    for c in range(NC):
        m = dcore == c
        sr, dl = srow[m], dloc[m]
        chunks = []
        for ch in range(NCHUNK):
            w0 = ch * WPC
            cm = (dl >= w0 * 128) & (dl < (w0 + WPC) * 128)
            s_c = sr[cm]
            d_c = dl[cm] - w0 * 128          # chunk-relative dst 0..895
            quads = {}
            for hname, padi in (("A", 0), ("B", HALF)):
                hm = (s_c < HALF) if hname == "A" else (s_c >= HALF)
                s_h, d_h = s_c[hm], d_c[hm]
                idxs = s_h + 1 if hname == "A" else s_h - HALF
                oo = np.lexsort((idxs, d_h))
                idxs, d_h = idxs[oo], d_h[oo]
                ql = []
                i = 0
                nh = len(d_h)
                while i < nh:
                    j = i
                    while j < nh and d_h[j] == d_h[i]:
                        j += 1
                    vals = idxs[i:j]
                    npadv = (-len(vals)) % 4
                    if npadv:
                        vals = np.concatenate(
                            [vals, np.full(npadv, padi, np.int64)])
                    for k in range(0, len(vals), 4):
                        ql.append((vals[k:k + 4], d_h[i]))
                    i = j
                quads[hname] = ql
            chunks.append(quads)
        per_core.append(chunks)

    meta = []
    for ch in range(NCHUNK):
        GA = max((len(per_core[c][ch]["A"]) + 127) // 128 for c in range(NC))
        GB = max((len(per_core[c][ch]["B"]) + 127) // 128 for c in range(NC))
        meta.append(dict(GA=max(GA, 1), GB=max(GB, 1)))
        meta[-1]["NG"] = meta[-1]["GA"] + meta[-1]["GB"]

    segs, idxs_np, scheds = [], [], []
    for ch, m in enumerate(meta):
        GA, GB, NG = m["GA"], m["GB"], m["NG"]
        m["cols"] = 8 * 5 * NG
        seg_c = np.full((NC, 128, NG), SENT, np.int16)
        idx_c = np.zeros((NC, 128, m["cols"]), np.int16)
        for c in range(NC):
            qa = list(per_core[c][ch]["A"])
            qb = list(per_core[c][ch]["B"])
            qa += [(np.full(4, 0, np.int64), -1)] * (GA * 128 - len(qa))
            qb += [(np.full(4, HALF, np.int64), -1)] * (GB * 128 - len(qb))
            allq = qa + qb
            xl_idx = np.zeros((4 * NG, 128), np.int64)
            xr_idx = np.zeros((NG, 128), np.int64)
            for qi, (vals, dv) in enumerate(allq):
                g, p = qi // 128, qi % 128
                xl_idx[4 * g:4 * g + 4, p] = vals
                if dv >= 0:
                    xr_idx[g, p] = dv + ch * WPC * 128
                    seg_c[c, p, g] = dv
            co = 0
            for t0 in range(0, 4 * NG, CAP):
                nt = min(CAP, 4 * NG - t0)
                idx_c[c, :, co:co + nt * 8] = _wrap16(
                    xl_idx[t0:t0 + nt].reshape(-1))
                co += nt * 8
            for t0 in range(0, NG, CAP):
                nt = min(CAP, NG - t0)
                idx_c[c, :, co:co + nt * 8] = _wrap16(
                    xr_idx[t0:t0 + nt].reshape(-1))
                co += nt * 8
        # SPMD-uniform matmul schedule
        win_of = [[] for _ in range(WPC)]
        for g in range(NG):
            sg = seg_c[:, :, g]
            valid = sg[sg != SENT]
            for w in np.unique(valid // 128):
                win_of[int(w)].append(g)
        for w in range(WPC):
            if not win_of[w]:
                win_of[w].append(0)
        sched = []
        for g in range(NG):
            for w in range(WPC):
                wl = win_of[w]
                if g in wl:
                    sched.append((g, w, g == wl[0], g == wl[-1]))
        segs.append(seg_c)
        idxs_np.append(idx_c)
        scheds.append(sched)
    return meta, segs, idxs_np, scheds


def _build(meta, scheds, rep=1):
    import concourse.bacc as bacc
    import concourse.mybir as mybir
    import concourse.tile as tile
    from concourse import library_config

    f16, f32, i16 = mybir.dt.float16, mybir.dt.float32, mybir.dt.int16

    nc = bacc.Bacc("TRN2", target_bir_lowering=False, debug=False,
                   num_devices=NC)
    d = {}
    sbuf_specs = [
        ("W1", [128, 512], f32), ("b1", [128, 512], f32),
        ("W2", [128, 256], f16), ("b2", [128, 128], f16),
        ("sg1", [128, 256], f16), ("sg2", [128, 64], f16),
        ("rc1", [128, 256], f16), ("rc2", [128, 64], f32),
        ("ba1", [128, 2], f32), ("ba2", [128, 64], f32),
        ("pr1", [1, 256], f16), ("pr2", [1, 128], f16),
        ("iot", [128, WPC * 128], i16),
    ]
    for ch, m in enumerate(meta):
        sbuf_specs.append((f"seg{ch}", [128, m["NG"]], i16))
    dram_specs = [("xT", [128, NPAD], f32)]
    for ch, m in enumerate(meta):
        dram_specs.append((f"idx{ch}", [128, m["cols"]], i16))
    for name, shape, dt in sbuf_specs + dram_specs:
        d[name] = nc.dram_tensor(name, shape, dt, kind="ExternalInput")
    out_d = nc.dram_tensor("out", [NPAD, 64], f32, kind="ExternalOutput")

    with tile.TileContext(nc) as tc:
        with (
            tc.tile_pool(name="const", bufs=1) as cp,
            tc.tile_pool(name="dram", bufs=1, space="DRAM") as dp,
        ):
            nc.gpsimd.load_library(library_config.mlp)
            T = {"out_d": out_d, "dp": dp, "xT_d": d["xT"]}
            for ch in range(len(meta)):
                T[f"idx{ch}_d"] = d[f"idx{ch}"]
            for name, shape, dt in sbuf_specs:
                tl = cp.tile(list(shape), dt, tag=name)
                nc.sync.dma_start(tl[:], d[name][:, :])
                T[name] = tl
            for r in range(rep):
                _one_pass(nc, tc, mybir, meta, scheds, T)
    nc.compile()
    return nc


def _one_pass(nc, tc, mybir, meta, scheds, T):
    f16, f32, i16 = mybir.dt.float16, mybir.dt.float32, mybir.dt.int16
    A = mybir.AluOpType
    AF = mybir.ActivationFunctionType
    dp = T["dp"]

    tab1 = dp.tile([NROWS + 2, 256], f16, tag="tab1")
    xl1_loc = dp.tile([NPAD, 256], f16, tag="xl1_loc")
    xr1_tab = dp.tile([NPAD, 256], f16, tag="xr1_tab")
    hraw = dp.tile([NPAD, 256], f16, tag="hraw")
    tab2 = dp.tile([NROWS + 2, 128], f16, tag="tab2")
    l2_loc = dp.tile([NPAD, 128], f16, tag="l2_loc")
    r2_tab = dp.tile([NPAD, 128], f16, tag="r2_tab")

    # ---------- node phase 1 + layer-1 self block ----------
    with tc.tile_pool(name="n1o", bufs=1) as n1o:
        rhs_self1 = n1o.tile([128, NWIN, 260], f16, tag="rhs_self1")
        with (
            tc.tile_pool(name="n1t", bufs=1) as n1t,
            tc.tile_pool(name="n1p", bufs=1, space="PSUM") as n1p,
        ):
            xlr = n1t.tile([128, NWIN, 512], f16, tag="xlr")
            for b in range(0, NWIN, 8):
                nb = min(8, NWIN - b)
                xw = n1t.tile([128, 8, 128], f32, tag="xw")
                nc.sync.dma_start(xw[:, 0:nb, :],
                                  T["xT_d"][:, b * 128:(b + nb) * 128]
                                  .rearrange("p (w q) -> p w q", q=128))
                ps = n1p.tile([128, 8, 512], f32, tag="psn1")
                for k in range(nb):
                    nc.tensor.matmul(ps[:, k, :], xw[:, k, :], T["W1"][:],
                                     start=True, stop=True,
                                     skip_group_check=True)
                nc.vector.tensor_tensor(
                    xlr[:, b:b + nb, :], ps[:, 0:nb, :],
                    T["b1"][:].unsqueeze(1).broadcast_to([128, nb, 512]), A.add)
            nc.sync.dma_start(
                xl1_loc[:, :].rearrange("(w p) c -> p w c", p=128),
                xlr[:, :, 0:256])
            nc.sync.dma_start(
                xr1_tab[:, :].rearrange("(w p) c -> p w c", p=128),
                xlr[:, :, 256:512])
            nc.sync.dma_start(tab1[0:1, :], T["pr1"][:])
            nc.sync.dma_start(tab1[NROWS + 1:NROWS + 2, :], T["pr1"][:])

            ms = n1t.tile([128, NWIN, 256], f16, tag="ms")
            nc.vector.tensor_tensor(ms[:], xlr[:, :, 0:256],
                                    xlr[:, :, 256:512], A.add)
            nc.vector.scalar_tensor_tensor(ms[:], ms[:], SLOPE, ms[:],
                                           A.mult, A.max)
            nc.vector.tensor_tensor(
                ms[:], ms[:],
                T["sg1"][:].unsqueeze(1).broadcast_to([128, NWIN, 256]),
                A.mult)
            es = n1t.tile([128, NWIN, 4], f32, tag="es")
            nc.vector.reduce_sum(
                es[:], ms[:].rearrange("p w (h c) -> p w h c", h=4),
                axis=mybir.AxisListType.X)
            nc.scalar.activation(rhs_self1[:, :, 0:4], es[:], AF.Exp)
            nc.vector.tensor_tensor(
                rhs_self1[:, :, 4:260].rearrange(
                    "p w (h c) -> p w h c", h=4),
                xlr[:, :, 0:256].rearrange("p w (h c) -> p w h c", h=4),
                rhs_self1[:, :, 0:4].unsqueeze(3).broadcast_to(
                    [128, NWIN, 4, 64]), A.mult)

        nc.gpsimd.collective_compute(
            "AllGather", A.bypass, replica_groups=[list(range(NC))],
            ins=[xl1_loc[:].opt()], outs=[tab1[1:NROWS + 1, :].opt()])

        _edge_layer(nc, tc, mybir, meta, scheds, T, layer=1,
                    tab=tab1, xr_tab=xr1_tab, rhs_self=rhs_self1,
                    hout=hraw)

    # ---------- node phase 2 + layer-2 self block ----------
    with tc.tile_pool(name="n2o", bufs=1) as n2o:
        rhs_self2 = n2o.tile([128, NWIN, 65], f16, tag="rhs_self2")
        with (
            tc.tile_pool(name="n2t", bufs=1) as n2t,
            tc.tile_pool(name="n2p", bufs=1, space="PSUM") as n2p,
        ):
            hT = n2t.tile([128, 2, NPAD], f16, tag="hT")
            for k in range(2):
                nc.sync.dma_start_transpose(
                    hT[:, k, :], hraw[:, k * 128:(k + 1) * 128])
            for k in range(2):
                nc.vector.tensor_scalar_add(hT[:, k, :], hT[:, k, :],
                                            T["ba1"][:, k:k + 1])
            lo = n2t.tile([128, 2, NPAD], f16, tag="lo")
            nc.vector.tensor_scalar_min(lo[:], hT[:], 0.0)
            nc.scalar.activation(lo[:], lo[:], AF.Exp)
            nc.vector.tensor_scalar_max(hT[:], hT[:], 0.0)
            nc.vector.scalar_tensor_tensor(hT[:], lo[:], -1.0, hT[:],
                                           A.add, A.add)

            l2r2 = n2t.tile([128, NWIN, 128], f16, tag="l2r2")
            for b in range(0, NWIN, 16):
                nb = min(16, NWIN - b)
                ps = n2p.tile([128, 16, 128], f32, tag="psn2")
                for k in range(nb):
                    for half in range(2):
                        nc.tensor.matmul(
                            ps[:, k, :],
                            hT[:, half, (b + k) * 128:(b + k + 1) * 128],
                            T["W2"][:, half * 128:(half + 1) * 128],
                            start=(half == 0), stop=(half == 1),
                            skip_group_check=True)
                nc.vector.tensor_tensor(
                    l2r2[:, b:b + nb, :], ps[:, 0:nb, :],
                    T["b2"][:].unsqueeze(1).broadcast_to([128, nb, 128]), A.add)
            nc.sync.dma_start(
                l2_loc[:, :].rearrange("(w p) c -> p w c", p=128)[:, :, 0:64],
                l2r2[:, :, 0:64])
            nc.sync.dma_start(
                r2_tab[:, :].rearrange("(w p) c -> p w c", p=128)[:, :, 0:64],
                l2r2[:, :, 64:128])
            nc.sync.dma_start(tab2[0:1, :], T["pr2"][:])
            nc.sync.dma_start(tab2[NROWS + 1:NROWS + 2, :], T["pr2"][:])

            ms2 = n2t.tile([128, NWIN, 64], f16, tag="ms2")
            nc.vector.tensor_tensor(ms2[:], l2r2[:, :, 0:64],
                                    l2r2[:, :, 64:128], A.add)
            nc.vector.scalar_tensor_tensor(ms2[:], ms2[:], SLOPE, ms2[:],
                                           A.mult, A.max)
            nc.vector.tensor_tensor(
                ms2[:], ms2[:],
                T["sg2"][:].unsqueeze(1).broadcast_to([128, NWIN, 64]),
                A.mult)
            es2 = n2t.tile([128, NWIN, 1], f32, tag="es2")
            nc.vector.reduce_sum(es2[:], ms2[:].unsqueeze(2),
                                 axis=mybir.AxisListType.X)
            nc.scalar.activation(rhs_self2[:, :, 0:1], es2[:], AF.Exp)
            nc.vector.tensor_tensor(
                rhs_self2[:, :, 1:65], l2r2[:, :, 0:64],
                rhs_self2[:, :, 0:1].broadcast_to([128, NWIN, 64]), A.mult)

        nc.gpsimd.collective_compute(
            "AllGather", A.bypass, replica_groups=[list(range(NC))],
            ins=[l2_loc[:].opt()], outs=[tab2[1:NROWS + 1, :].opt()])

        _edge_layer(nc, tc, mybir, meta, scheds, T, layer=2,
                    tab=tab2, xr_tab=r2_tab, rhs_self=rhs_self2,
                    hout=None)


def _edge_layer(nc, tc, mybir, meta, scheds, T, layer, tab, xr_tab,
                rhs_self, hout):
    f16, f32, i16 = mybir.dt.float16, mybir.dt.float32, mybir.dt.int16
    A = mybir.AluOpType
    AF = mybir.ActivationFunctionType
    CH = 256 if layer == 1 else 128     # gathered row width
    CV = 256 if layer == 1 else 64      # valid channels
    NH = 4 if layer == 1 else 1
    CPH = CV // NH
    RW = NH + CV
    sg = T["sg1"] if layer == 1 else T["sg2"]
    tg = f"L{layer}"

    with (
        tc.tile_pool(name=tg, bufs=1) as pool,
        tc.tile_pool(name=tg + "p", bufs=1, space="PSUM") as ppool,
    ):
        for ch, m in enumerate(meta):
            NG = m["NG"]
            NT = 4 * NG
            idxs = pool.tile([128, m["cols"]], i16, tag=tg + "ix")
            nc.sync.dma_start(idxs[:], T[f"idx{ch}_d"][:, :])
            xlg = pool.tile([128, NT, CH], f16, tag=tg + "xl")
            xrg = pool.tile([128, NG, CH], f16, tag=tg + "xr")
            co = 0
            nA = 4 * m["GA"]
            for lo_t, hi_t, srcap in (
                (0, nA, tab[0:HALF + 1, :]),
                (nA, NT, tab[HALF + 1:NROWS + 2, :]),
            ):
                t0 = lo_t
                while t0 < hi_t:
                    ntc = min(CAP, hi_t - t0)
                    nc.gpsimd.dma_gather(
                        xlg[:, t0:t0 + ntc, :], srcap,
                        idxs[:, co:co + ntc * 8],
                        ntc * 128, ntc * 128, CH, single_packet=False)
                    co += ntc * 8
                    t0 += ntc
            t0 = 0
            while t0 < NG:
                ntc = min(CAP, NG - t0)
                nc.gpsimd.dma_gather(
                    xrg[:, t0:t0 + ntc, :], xr_tab[:, :],
                    idxs[:, co:co + ntc * 8],
                    ntc * 128, ntc * 128, CH, single_packet=False)
                co += ntc * 8
                t0 += ntc

            xl3 = xlg[:] if layer == 1 else xlg[:, :, 0:64]
            xr3 = xrg[:] if layer == 1 else xrg[:, :, 0:64]
            mt = pool.tile([128, NT, CV], f16, tag=tg + "m")
            nc.vector.tensor_tensor(
                mt[:].rearrange("p (g j) c -> p g j c", j=4),
                xl3.rearrange("p (g j) c -> p g j c", j=4),
                xr3.unsqueeze(2).broadcast_to([128, NG, 4, CV]), A.add)
            nc.vector.scalar_tensor_tensor(mt[:], mt[:], SLOPE, mt[:],
                                           A.mult, A.max)
            nc.vector.tensor_tensor(
                mt[:], mt[:],
                sg[:].unsqueeze(1).broadcast_to([128, NT, CV]), A.mult)
            e = pool.tile([128, NT, NH], f32, tag=tg + "e")
            nc.vector.reduce_sum(
                e[:], mt[:].rearrange("p t (h c) -> p t h c", h=NH),
                axis=mybir.AxisListType.X)
            w = pool.tile([128, NT, NH], f16, tag=tg + "w")
            nc.scalar.activation(w[:], e[:], AF.Exp)
            nc.vector.tensor_tensor(
                xl3.rearrange("p t (h c) -> p t h c", h=NH),
                xl3.rearrange("p t (h c) -> p t h c", h=NH),
                w[:].unsqueeze(3).broadcast_to([128, NT, NH, CPH]), A.mult)
            # quad reduction (2 levels); level-1 scratch reuses mt
            rhs = pool.tile([128, NG, RW], f16, tag=tg + "rhs")
            l1w = pool.tile([128, NG, 2, NH], f16, tag=tg + "l1w")
            wv = w[:].rearrange("p (g u j) h -> p g u j h", u=2, j=2)
            nc.vector.tensor_tensor(l1w[:], wv[:, :, :, 0, :],
                                    wv[:, :, :, 1, :], A.add)
            nc.vector.tensor_tensor(rhs[:, :, 0:NH], l1w[:, :, 0, :],
                                    l1w[:, :, 1, :], A.add)
            xv = xl3.rearrange("p (g u j) c -> p g u j c", u=2, j=2)
            mv = mt[:].rearrange("p (g u) c -> p g u c", u=4)
            nc.vector.tensor_tensor(mv[:, :, 0:2, :], xv[:, :, :, 0, :],
                                    xv[:, :, :, 1, :], A.add)
            nc.vector.tensor_tensor(rhs[:, :, NH:RW], mv[:, :, 0, :],
                                    mv[:, :, 1, :], A.add)
            ind = pool.tile([128, NG, WPC * 128], f16, tag=tg + "ind")
            nc.vector.tensor_tensor(
                ind[:],
                T["iot"][:].unsqueeze(1).broadcast_to([128, NG, WPC * 128]),
                T[f"seg{ch}"][:, :].unsqueeze(2)
                .broadcast_to([128, NG, WPC * 128]), A.is_equal)

            ps = ppool.tile([128, WPC, 512], f32, tag=tg + "ps",
                            name=f"{tg}ps{ch}")
            for (g, wn, st, sp) in scheds[ch]:
                nc.tensor.matmul(
                    ps[:, wn, 0:RW], ind[:, g, wn * 128:(wn + 1) * 128],
                    rhs[:, g, :], start=st, stop=sp, skip_group_check=True)

            P = pool.tile([128, WPC, RW], f32, tag=tg + "P")
            nc.scalar.copy(P[:], ps[:, :, 0:RW])
            nc.vector.tensor_tensor(
                P[:], P[:], rhs_self[:, ch * WPC:(ch + 1) * WPC, :], A.add)
            rec = pool.tile([128, WPC, NH], f32, tag=tg + "rec")
            nc.vector.reciprocal(rec[:], P[:, :, 0:NH])
            if layer == 1:
                h16 = pool.tile([128, WPC, 256], f16, tag=tg + "h16")
                nc.vector.tensor_tensor(
                    h16[:].rearrange("p w (h c) -> p w h c", h=4),
                    P[:, :, 4:260].rearrange("p w (h c) -> p w h c", h=4),
                    rec[:].unsqueeze(3).broadcast_to([128, WPC, 4, 64]),
                    A.mult)
                nc.vector.tensor_tensor(
                    h16[:], h16[:],
                    T["rc1"][:].unsqueeze(1).broadcast_to([128, WPC, 256]), A.mult)
                nc.sync.dma_start(
                    hout[ch * WPC * 128:(ch + 1) * WPC * 128, :]
                    .rearrange("(w p) c -> p w c", p=128), h16[:])
            else:
                o32 = pool.tile([128, WPC, 64], f32, tag=tg + "o32")
                nc.vector.tensor_tensor(
                    o32[:], P[:, :, 1:65],
                    rec[:].broadcast_to([128, WPC, 64]), A.mult)
                nc.vector.tensor_tensor(
                    o32[:], o32[:],
                    T["rc2"][:].unsqueeze(1).broadcast_to([128, WPC, 64]), A.mult)
                nc.vector.tensor_tensor(
                    o32[:], o32[:],
                    T["ba2"][:].unsqueeze(1).broadcast_to([128, WPC, 64]), A.add)
                nc.sync.dma_start(
                    T["out_d"][ch * WPC * 128:(ch + 1) * WPC * 128, :]
                    .rearrange("(w p) c -> p w c", p=128), o32[:])


def _host_prep(inputs):
    att1 = np.asarray(inputs["att1"], np.float64)
    att2 = np.asarray(inputs["att2"], np.float64)[0]
    f1 = np.maximum(np.abs(att1.reshape(-1)), 1e-30)
    s1 = np.where(att1.reshape(-1) >= 0, 1.0, -1.0)
    f2 = np.maximum(np.abs(att2), 1e-30)
    s2 = np.where(att2 >= 0, 1.0, -1.0)

    W1 = np.concatenate([np.asarray(inputs["Wl1"], np.float64) * f1,
                         np.asarray(inputs["Wr1"], np.float64) * f1], 1)
    b1 = np.concatenate([np.asarray(inputs["bl1"], np.float64) * f1,
                         np.asarray(inputs["br1"], np.float64) * f1])
    W2c = np.concatenate([np.asarray(inputs["Wl2"], np.float64) * f2,
                          np.asarray(inputs["Wr2"], np.float64) * f2], 1)
    W2 = np.concatenate([W2c[0:128], W2c[128:256]], 1)   # [128, 256]
    b2 = np.concatenate([np.asarray(inputs["bl2"], np.float64) * f2,
                         np.asarray(inputs["br2"], np.float64) * f2])

    com = dict(
        W1=W1.astype(np.float32), b1=np.tile(b1.reshape(1, 512), (128, 1)).astype(np.float32),
        W2=W2.astype(np.float16), b2=np.tile(b2.reshape(1, 128), (128, 1)).astype(np.float16),
        sg1=np.tile(s1.astype(np.float16), (128, 1)),
        sg2=np.tile(s2.astype(np.float16), (128, 1)),
        rc1=np.tile((1.0 / f1).reshape(1, 256), (128, 1)).astype(np.float16),
        rc2=np.tile((1.0 / f2).reshape(1, 64), (128, 1)).astype(np.float32),
        ba1=np.asarray(inputs["bias1"], np.float64).reshape(2, 128).T
        .astype(np.float32).copy(),
        ba2=np.tile(np.asarray(inputs["bias2"], np.float32).reshape(1, 64), (128, 1)),
        pr1=(-1e4 * s1).reshape(1, 256).astype(np.float16),
        pr2=np.concatenate([-1e4 * s2, np.zeros(64)]).reshape(1, 128)
        .astype(np.float16),
        iot=np.tile(np.arange(WPC * 128, dtype=np.int16), (128, 1)),
    )
    x = np.asarray(inputs["x"], np.float32)
    xTs = []
    for c in range(NC):
        xt = np.zeros((128, NPAD), np.float32)
        xt[:, 0:NLOC] = x[c * NLOC:(c + 1) * NLOC].T
        xTs.append(xt)
    return com, xTs


def _get_built(edge_index, rep=1):
    key = (hash(np.asarray(edge_index).tobytes()), rep)
    if key not in _cache:
        meta, segs, idxs_np, scheds = _host_metadata(edge_index)
        nc = _build(meta, scheds, rep=rep)
        _cache[key] = (meta, segs, idxs_np, scheds, nc)
    return _cache[key]


def make_maps(inputs, meta, segs, idxs_np):
    com, xTs = _host_prep(inputs)
    maps = []
    for c in range(NC):
        mp = dict(com)
        mp["xT"] = xTs[c]
        for ch in range(len(meta)):
            mp[f"seg{ch}"] = segs[ch][c]
            mp[f"idx{ch}"] = idxs_np[ch][c]
        maps.append(mp)
    return maps


def kernel(**inputs):
    from concourse.bass_utils import run_bass_kernel_spmd

    meta, segs, idxs_np, scheds, nc = _get_built(inputs["edge_index"])
    maps = make_maps(inputs, meta, segs, idxs_np)
    res = run_bass_kernel_spmd(nc, maps, list(range(NC)))
    out = np.zeros((N, OUT), np.float32)
    for c in range(NC):
        out[c * NLOC:(c + 1) * NLOC] = res.results[c]["out"][0:NLOC]
    return out
